# revision 1
# baseline (speedup 1.0000x reference)
"""MoE routed matmul on 8 NeuronCores (Trainium2, Bass).

Problem: out[b] = x[b] @ W[idx[b]]  with  x:(2048,256), W:(64,256,256),
idx:(2048,1) int32.

Strategy: expert-parallel. Experts (contexts) are sharded 8-per-core.
The host routes samples to the core that owns their expert (this is the
all-to-all, done during input sharding), padding each expert's sample
group to a fixed capacity CAP so the SPMD device program is fully
static. Each core then does 8 dense (CAP x 256) @ (256 x 256) matmuls —
weights are read from HBM exactly once across the whole device, which is
what the memory-bound roofline wants. The host scatters the device
output back to the original sample order.

v4 over the f32 baseline (all validated against the CoreSim cost model,
which matches the graded HW exec time almost exactly):
  - fp16 end-to-end on device (x, W, out). PSUM accumulates f32. The
    routed-matmul output error vs the f32 reference is ~5e-4 relative,
    far under the 2e-2 gate, and it halves every DMA byte. It also takes
    the matmuls off the PE's slow fp32 path (4 cycles/row -> 1).
  - CAP=48 (max expert count for the problem's routing is 45; the
    pair-per-bank PSUM packing supports any even CAP <= 64).
  - Host pre-permutes xt and w into per-partition-linear DRAM layouts so
    every input DMA is a pure [128, N] linear block transfer; weights
    stream in 4 groups of 2 experts so matmuls chase the DMA stream.
  - One PSUM bank per expert pair (partitions 0:CAP and 64:64+CAP), so
    eviction is one wide [112, 256] convert-copy per pair (DVE cost
    scales with free-dim size only, so wide+narrow beats narrow+tall 8x).
  - The device out tensor keeps the 16-row PSUM partition hole (memset
    once in the preamble), making each pair's writeback a single fully
    contiguous [112, 256] DMA; the host skips the junk rows. Writeback
    DMAs alternate between the scalar and sync queues so their ~500 ns
    issue slices overlap.

niter > 1 replicates the body with double-buffered inputs and WAR
semaphore chaining — used by the benchmark harness to measure
steady-state per-iteration HW time via wall-clock slope.
"""

import numpy as np
from contextlib import ExitStack

B, D, U, C = 2048, 256, 256, 64
NCORES = 8
EPC = C // NCORES  # experts per core
CAP = 48           # per-expert sample capacity (padded); max count is 45

_prog_cache: dict = {}


def _build_program(cap: int, niter: int = 1, wgroup=None, warmup: int = 0,
                   serial: bool = False, stage: str = "full"):
    import concourse.bass as bass
    from concourse import mybir
    from concourse.bass import compact_to_ranges

    f16 = mybir.dt.float16
    f32 = mybir.dt.float32
    assert cap % 2 == 0 and cap <= 64
    # wgroup: weight-DMA group sizes (experts per DMA); an int means
    # uniform groups. Tapered groupings sim identically to uniform 2.
    if wgroup is None:
        wgroup = 2
    if isinstance(wgroup, int):
        wgroup = (wgroup,) * (EPC // wgroup)
    assert sum(wgroup) == EPC and EPC == 8
    npair = EPC // 2
    ngrp = len(wgroup)
    goff = [sum(wgroup[:g]) for g in range(ngrp)]          # first expert of g
    gof = {}
    for g in range(ngrp):
        for e in range(wgroup[g]):
            gof[goff[g] + e] = (g, e)
    nc = bass.Bass()
    # xt: x^T, host-prepacked [p, k, c] so the DMA is [128, 2*EPC*cap] linear
    xt = nc.declare_dram_parameter("xt", [128, 2 * EPC * cap], f16, isOutput=False)
    # w: host-prepacked [p, e, k, u] — per-partition linear, so any
    # contiguous expert range is one linear DMA slice
    w = nc.declare_dram_parameter("w", [128, EPC * 2 * U], f16, isOutput=False)
    # device out rows per pair: expert 2p at rows 0:cap, 16 junk rows
    # (cap:64, the PSUM hole), expert 2p+1 at rows 64:64+cap. Shipping the
    # hole keeps the pair writeback a single fully-contiguous DMA; the host
    # skips the junk rows when unsharding.
    out = nc.declare_dram_parameter("out", [EPC // 2, 64 + cap, U], f16,
                                    isOutput=True)

    NSET = 2 if niter > 1 else 1

    with ExitStack() as ctx:
        # xt SBUF: [128, 2, EPC*cap] — the two K-chunks in a free dim
        sb_xt = [
            ctx.enter_context(nc.sbuf_tensor(f"sb_xt{s}", [128, 2, EPC * cap], f16))
            for s in range(NSET)
        ]
        # w SBUF per DMA group: [128, group_size, 2, U]
        sb_w = [
            [
                ctx.enter_context(
                    nc.sbuf_tensor(f"sb_w{g}_{s}", [128, wgroup[g], 2, U], f16)
                )
                for s in range(NSET)
            ]
            for g in range(ngrp)
        ]
        # per-pair staging, mirroring the PSUM packing
        sb_out = [
            ctx.enter_context(nc.sbuf_tensor(f"sb_out{p}", [128, U], f16))
            for p in range(npair)
        ]
        # one PSUM bank per expert pair: expert 2p+e at partitions
        # e*64 .. e*64+cap, columns 0:U
        ps = [
            ctx.enter_context(nc.psum_tensor(f"ps{p}", [128, 512], f32))
            for p in range(npair)
        ]
        if warmup:
            sb_warm = ctx.enter_context(nc.sbuf_tensor("sb_warm", [128, 512], f16))
            ps_warm = ctx.enter_context(nc.psum_tensor("ps_warm", [128, 512], f32))

        # Dedicated sems per buffer group: a wait threshold on a sem that
        # counts several in-flight DMAs is unsound (a DMA's +16 completion
        # is split +1 across 16 SDMA engines, so a later DMA's increments
        # can satisfy an earlier DMA's threshold while it still has a
        # straggler engine). One sem per buffer makes thresholds exact.
        warm_sem = ctx.enter_context(nc.semaphore("warm_sem"))
        hole_sem = ctx.enter_context(nc.semaphore("hole_sem"))
        xt_sem = ctx.enter_context(nc.semaphore("xt_sem"))
        w_sem = [ctx.enter_context(nc.semaphore(f"w_sem{g}")) for g in range(ngrp)]
        mm_sem = ctx.enter_context(nc.semaphore("mm_sem"))
        cp_sem = ctx.enter_context(nc.semaphore("cp_sem"))
        out_sem = [ctx.enter_context(nc.semaphore(f"out_sem{p}")) for p in range(npair)]

        # Semaphores are NOT cleared when a loaded NEFF is re-executed, so
        # absolute wait thresholds would be stale on the second run. Clear
        # the whole kernel sem range up front (same preamble the BIR
        # lowering path emits), then a pseudo-sync barrier keeps every
        # engine parked until the clears retire.
        for sem_range in compact_to_ranges(
            [s for s in nc._kernel_sem_range if s not in nc.barrier_sems]
        ):
            nc.gpsimd.dma_reset(sem_range)
            nc.gpsimd.sem_clear(sem_range)
        nc._nrt_pseudo_barrier()
        if warmup:
            nc.gpsimd.memset(sb_warm[:, :], 0.0).then_inc(warm_sem, 1)

        block = ctx.enter_context(nc.Block())

        def issue_out(eng, i, p):
            eng.wait_ge(cp_sem, npair * i + p + 1)
            eng.dma_start(out[p], sb_out[p][0:64 + cap, :]).then_inc(
                out_sem[p], 16)

        @block.sync
        def _(sync):
            for i in range(niter):
                s = i % NSET
                if serial and i >= 1:
                    # benchmark mode: no cross-iteration overlap, so each
                    # iteration behaves like an isolated cold call
                    if stage == "dma":
                        sync.wait_ge(w_sem[ngrp - 1], 16 * i)
                    elif stage == "dmamm":
                        sync.wait_ge(mm_sem, EPC * i)
                    else:
                        for p in range(npair):
                            sync.wait_ge(out_sem[p], 16 * i)
                if i >= 2:
                    # xt set s was read by all matmuls of iter i-2
                    sync.wait_ge(mm_sem, EPC * (i - 1))
                sync.dma_start(sb_xt[s][:, :, :], xt[:, :]).then_inc(xt_sem, 16)
                for g in range(ngrp):
                    if i >= 2:
                        # last expert of group g, iter i-2, done
                        sync.wait_ge(mm_sem,
                                     EPC * (i - 2) + goff[g] + wgroup[g])
                    src_g = w[:, goff[g] * 2 * U:(goff[g] + wgroup[g]) * 2 * U]
                    sync.dma_start(sb_w[g][s][:, :, :, :], src_g).then_inc(
                        w_sem[g], 16)
                if stage == "full":
                    # odd pairs' writeback issues from here: the sync queue is
                    # idle once the inputs are away, and two issuing engines
                    # halve the serialized out-DMA issue chain on the tail
                    for p in (1, 3):
                        issue_out(sync, i, p)
            if stage == "full":
                for p in (1, 3):
                    sync.wait_ge(out_sem[p], 16 * niter)
            if stage == "dma":
                # stripped bench variant: nothing downstream consumes the
                # input sems, so quiesce the DMAs before the program ends
                sync.wait_ge(xt_sem, 16 * niter)
                for g in range(ngrp):
                    sync.wait_ge(w_sem[g], 16 * niter)

        @block.tensor
        def _(tensor):
            if stage == "dma":
                return
            if warmup:
                tensor.wait_ge(warm_sem, 1)
            for i in range(niter):
                if warmup:
                    # Dummy matmuls: sustained PE activity releases the HAM
                    # clock gate (1.2 -> 2.4 GHz) while input DMAs stream, so
                    # the real matmuls run at full rate even in a cold call.
                    for _ in range(warmup):
                        tensor.matmul(
                            ps_warm[:, :], sb_warm[:, 0:128], sb_warm[:, :],
                            start=True, stop=True,
                        )
                s = i % NSET
                for j in range(EPC):
                    p, half = j // 2, j % 2
                    g, e_local = gof[j]
                    if j == 0:
                        tensor.wait_ge(xt_sem, 16 * (i + 1))
                    if e_local == 0:
                        tensor.wait_ge(w_sem[g], 16 * (i + 1))
                    if i == 0 and half == 0:
                        # bank p's hole memzero double-writes rows 32:cap;
                        # fires before this pair's weights land, never blocks
                        tensor.wait_ge(hole_sem, p + 1)
                    if i >= 1 and stage == "full" and half == 0:
                        # pair bank p was copied out during iter i-1
                        tensor.wait_ge(cp_sem, npair * (i - 1) + p + 1)
                    for k in range(2):
                        mm = tensor.matmul(
                            ps[p][half * 64:half * 64 + cap, 0:U],
                            sb_xt[s][:, k, j * cap:(j + 1) * cap],
                            sb_w[g][s][:, e_local, k, :],
                            start=(k == 0),
                            stop=(k == 1),
                        )
                    mm.then_inc(mm_sem, 1)

        @block.vector
        def _(vector):
            if stage in ("dma", "dmamm"):
                return
            # Initialize the dead partition rows cap:64 of each pair bank
            # once (GPSIMD cannot access PSUM, so this runs here; program
            # order on the vector engine makes it race-free), so the wide
            # per-pair copies never read uninitialized PSUM. Rows
            # 64+cap:128 are never read.
            for i in range(niter):
                for p in range(npair):
                    # pair copy: ready as soon as the pair's matmuls land
                    vector.wait_ge(mm_sem, EPC * i + 2 * (p + 1))
                    if i == 0:
                        # race-detector edge; fires long before the copy
                        vector.wait_ge(hole_sem, p + 1)
                    if i >= 1:
                        vector.wait_ge(out_sem[p], 16 * i)
                    vector.tensor_copy(
                        sb_out[p][0:64 + cap, :],
                        ps[p][0:64 + cap, 0:U],
                    ).then_inc(cp_sem, 1)

        @block.scalar
        def _(scalar):
            if stage in ("dma", "dmamm"):
                return
            # Initialize the dead partition rows cap:64 of each pair bank
            # once, so the wide per-pair copies never read uninitialized
            # PSUM. On this engine (idle until writeback) to keep the DVE
            # and PE queues untouched; PSUM engine accesses need partition
            # base/count aligned to 32, so clear 32:64 — rows 32:cap are
            # re-written by the matmuls (hole_sem edge orders that).
            for p in range(npair):
                scalar.memzero(ps[p][32:64, 0:U]).then_inc(hole_sem, 1)
            for i in range(niter):
                for p in (0, 2):
                    issue_out(scalar, i, p)
            for p in (0, 2):
                scalar.wait_ge(out_sem[p], 16 * niter)

    return nc


def _route(content_idx: np.ndarray, x: np.ndarray, cap: int):
    """Sort samples by expert; compute per-core padded packed-x shards.

    Returns xt_all in the device DMA layout [NCORES, 128, 2, EPC*cap]
    (partition p = d % 128, K-chunk k = d // 128), fp16.
    """
    idx = content_idx.reshape(-1).astype(np.int64)
    order = np.argsort(idx, kind="stable")
    e_sorted = idx[order]
    counts = np.bincount(idx, minlength=C)
    while counts.max() > cap:
        cap *= 2
    start = np.zeros(C, dtype=np.int64)
    start[1:] = np.cumsum(counts)[:-1]
    slot = np.arange(B) - start[e_sorted]
    core = e_sorted // EPC
    jl = e_sorted % EPC
    # xt columns are in local-expert order (matmul j reads block j)
    xcol = jl * cap + slot
    # device out rows: pair p = jl//2 occupies a (64+cap)-row block with
    # expert 2p at offset 0, 16 junk rows, expert 2p+1 at offset 64
    ocol = (jl // 2) * (64 + cap) + (jl % 2) * 64 + slot

    xt_all = np.zeros((NCORES, 128, 2, EPC * cap), dtype=np.float16)
    # sample vector (256,) -> [k, p] -> transpose to [p, k]
    xs = x[order].astype(np.float16).reshape(B, 2, 128).transpose(0, 2, 1)
    xt_all[core, :, :, xcol] = xs
    return cap, order, core, ocol, xt_all


def _unshard(outs: np.ndarray, order, core, col, cap: int) -> np.ndarray:
    """Scatter per-core padded device output back to original sample order."""
    outs = outs.reshape(NCORES, -1, U)
    out_full = np.empty((B, U), dtype=np.float32)
    out_full[order] = outs[core, col, :].astype(np.float32)
    return out_full


def _make_in_maps(xt_all: np.ndarray, kernel_w: np.ndarray, wgroup=None):
    # [C, D, U] -> [NC, EPC, 2, 128, U] -> [NC, 128, (e k u)] — grouping-
    # independent per-partition-linear layout
    w = np.ascontiguousarray(
        kernel_w.astype(np.float16)
        .reshape(NCORES, EPC, 2, 128, U)
        .transpose(0, 3, 1, 2, 4)
        .reshape(NCORES, 128, EPC * 2 * U)
    )
    xt = xt_all.reshape(NCORES, 128, -1)
    return [{"xt": xt[c], "w": w[c]} for c in range(NCORES)]


def kernel(content_idx: np.ndarray, x: np.ndarray, kernel: np.ndarray) -> np.ndarray:
    from concourse.bass_utils import run_bass_kernel_spmd

    cap, order, core, col, xt_all = _route(content_idx, x, CAP)
    if cap > CAP:
        # Pathologically skewed routing (an expert holds >CAP samples) can't
        # use the static packed program. Unreachable for the fixed-seed
        # problem data; fall back to a host computation to stay correct.
        idx = content_idx.reshape(-1).astype(np.int64)
        return np.einsum("bd,bdu->bu", x.astype(np.float32),
                         kernel.astype(np.float32)[idx]).astype(np.float32)

    key = (cap, 1)
    if key not in _prog_cache:
        _prog_cache[key] = _build_program(cap, 1)
    nc = _prog_cache[key]

    in_maps = _make_in_maps(xt_all, kernel)
    res = run_bass_kernel_spmd(nc, in_maps, list(range(NCORES)))
    outs = np.stack([res.results[c]["out"] for c in range(NCORES)])
    return _unshard(outs, order, core, col, cap)



# revision 28
# speedup vs baseline: 1.4223x; 1.4223x over previous
"""MoE routed matmul on 8 NeuronCores (Trainium2, Bass).

Problem: out[b] = x[b] @ W[idx[b]]  with  x:(2048,256), W:(64,256,256),
idx:(2048,1) int32.

Strategy: expert-parallel. Experts (contexts) are sharded 8-per-core.
The host routes samples to the core that owns their expert (the
all-to-all, done during input sharding), padding each expert's sample
group to a fixed capacity CAP=48 so the SPMD device program is fully
static. The host scatters the device output back to original order.

v7 over the v4 baseline (7642 ns):
  - TRANSPOSED matmuls: out^T[u, s] = sum_d w[d, u] * x^T[d, s] with the
    UNITS dim on PSUM partitions and the (padded) samples on the free
    dim. Matmul and PSUM-eviction cost scale with the output free size,
    so 48-wide transposed panels are ~5x cheaper than 256-wide ones
    (32 matmuls x ~40 ns instead of 16 x ~107-213 ns; 16 copies x ~75 ns
    instead of 4 x 392 ns).
  - Input DMAs spread across all three DMA-capable queues (SP, Act,
    Pool); same-queue transfers serialize but cross-queue ones overlap,
    so the input phase is ~1.6 us instead of 3.7 us serialized.
  - PE paces itself with cheap dummy matmuls sized so its first real
    semaphore wait ARRIVES AFTER the input DMA completes: a blocked
    wait on a DMA semaphore eats a ~1.7 us wake-up penalty, a wait that
    arrives late passes instantly. The weight stream is ordered so every
    later expert's DMA lands before the PE reaches it.
  - All evictions on DVE (engine-sem waits wake in 100 ns); Activation
    is a pure DMA queue (no act-table load anywhere).
  - Two output DMAs (one per u-chunk bank) issue as soon as their bank's
    last panel is evicted; program end then rides the DMA's own
    completion chain (~1.9 us, irreducible).
"""

import numpy as np
from contextlib import ExitStack

B, D, U, C = 2048, 256, 256, 64
NCORES = 8
EPC = C // NCORES   # experts per core
CAP = 46            # per-expert sample capacity (padded); max count is 45
SAMP = EPC * CAP    # padded sample columns per core

# PE dummy-matmul count: paces the PE so its first real wait arrives just
# after the xt/w0 DMA semaphores are already satisfied (~1.2 us in).
NDUMMY = 17
# DVE pacing copies: the first eviction's wait should also arrive late
# (a parked engine-sem wait wakes 100 ns after the matmuls finish).
NDVE = 15
_prog_cache: dict = {}


def _build_program(cap: int, niter: int = 1):
    import concourse.bass as bass
    from concourse import mybir
    from concourse.bass import compact_to_ranges

    assert niter == 1
    f16 = mybir.dt.float16
    f32 = mybir.dt.float32
    samp = EPC * cap

    nc = bass.Bass()
    # xt: x^T host-prepacked [p, k*samp + e*cap + slot]; each k half is a
    # single [128, samp] linear block
    xt = nc.declare_dram_parameter("xt", [128, 2 * samp], f16, isOutput=False)
    # w: host-prepacked [p, ((e*2)+k)*U + u]; one expert = 1024 linear cols
    w = nc.declare_dram_parameter("w", [128, EPC * 2 * U], f16, isOutput=False)
    # out: transposed [u-chunk b, u-within-chunk (partition), e*cap + slot]
    out = nc.declare_dram_parameter("out", [2, 128, samp], f16, isOutput=True)

    with ExitStack() as ctx:
        sb_xt = ctx.enter_context(nc.sbuf_tensor("sb_xt", [128, 2, samp], f16))
        sb_w = [
            ctx.enter_context(nc.sbuf_tensor(f"sb_w{g}", [128, 2, 2, U], f16))
            for g in range(4)
        ]
        sb_out = [
            ctx.enter_context(nc.sbuf_tensor(f"sb_out{b}", [128, samp], f16))
            for b in range(2)
        ]
        sb_scr = ctx.enter_context(nc.sbuf_tensor("sb_scr", [128, 96], f16))
        # 2 quad-sets of 2 banks (one per u-chunk): experts 4q+i write
        # columns i*cap of set q's banks. 4-expert-wide evictions amortize
        # the ~125 ns PSUM access latency per DVE op, and with one set per
        # quad there is no bank reuse, so the PE never waits on evictions.
        ps = [
            [
                ctx.enter_context(nc.psum_tensor(f"ps{q}_{b}", [128, 512], f32))
                for b in range(2)
            ]
            for q in range(2)
        ]
        ps_scr = ctx.enter_context(nc.psum_tensor("ps_scr", [128, 512], f32))

        # one sem per DMA buffer so +16 thresholds are exact
        xt_sem = ctx.enter_context(nc.semaphore("xt_sem"))
        w_sem = [ctx.enter_context(nc.semaphore(f"w_sem{g}")) for g in range(4)]
        scr_sem = ctx.enter_context(nc.semaphore("scr_sem"))
        mm_sem = ctx.enter_context(nc.semaphore("mm_sem"))
        cp_sem = ctx.enter_context(nc.semaphore("cp_sem"))
        out_sem = [ctx.enter_context(nc.semaphore(f"out_sem{b}")) for b in range(2)]

        # Clear the kernel sem range (sems persist across NEFF re-executions),
        # then barrier so no engine races the clears.
        for sem_range in compact_to_ranges(
            [s for s in nc._kernel_sem_range if s not in nc.barrier_sems]
        ):
            nc.gpsimd.dma_reset(sem_range)
            nc.gpsimd.sem_clear(sem_range)
        nc._nrt_pseudo_barrier()

        block = ctx.enter_context(nc.Block())

        def dma_w(eng, g):
            # group g = experts 2g, 2g+1: one [128, 2048 B/partition] block
            eng.dma_start(
                sb_w[g][:, :, :, :], w[:, g * 4 * U:(g + 1) * 4 * U]
            ).then_inc(w_sem[g], 16)

        # Queue plan (same-queue DMA slots serialize; cross-queue overlap;
        # every slot has a ~500 ns floor):
        #   SP:   xt [400..990], w45 [990..1780], out b0
        #   Act:  w01 [400..1190], w67 [1190..1980], out b1
        #   Pool: w23 [400..~1290]
        #   DVE:  all 16 panel evictions (e0b0, e0b1, e1b0, ...)
        #   PE:   NDUMMY pace matmuls, then 4 matmuls per expert

        @block.sync
        def _(sync):
            sync.dma_start(sb_xt[:, :, :], xt[:, :]).then_inc(xt_sem, 16)
            dma_w(sync, 2)
            sync.wait_ge(cp_sem, 3)
            sync.dma_start(out[0], sb_out[0][:, :]).then_inc(out_sem[0], 16)
            sync.wait_ge(out_sem[0], 16)

        @block.scalar
        def _(scalar):
            dma_w(scalar, 0)
            dma_w(scalar, 3)
            scalar.wait_ge(cp_sem, 4)
            scalar.dma_start(out[1], sb_out[1][:, :]).then_inc(out_sem[1], 16)
            scalar.wait_ge(out_sem[1], 16)

        @block.gpsimd
        def _(g):
            g.memset(sb_scr[:, :], 0.0).then_inc(scr_sem, 1)
            dma_w(g, 1)

        @block.tensor
        def _(tensor):
            tensor.wait_ge(scr_sem, 1)
            for i in range(NDUMMY):
                # ~40 ns each (48-wide output): pace, and keep the PE busy
                tensor.matmul(
                    ps_scr[0:8, 0:48], sb_scr[:, 0:8], sb_scr[:, 16:64],
                    start=True, stop=True,
                )
            tensor.wait_ge(xt_sem, 16)
            # quad q, u-chunk b, expert 4q+i: all four b0 panels of a quad,
            # then its four b1 panels — the quad's b panel strip completes
            # as one contiguous [128, 4*cap] PSUM region per bank
            for q in range(2):
                for b in range(2):
                    for i in range(4):
                        e = 4 * q + i
                        if b == 0 and e % 2 == 0:
                            tensor.wait_ge(w_sem[e // 2], 16)
                        for k in range(2):
                            mm = tensor.matmul(
                                ps[q][b][:, i * cap:(i + 1) * cap],
                                sb_w[e // 2][:, e % 2, k, b * 128:(b + 1) * 128],
                                sb_xt[:, k, e * cap:(e + 1) * cap],
                                start=(k == 0),
                                stop=(k == 1),
                            )
                        mm.then_inc(mm_sem, 1)

        @block.vector
        def _(vector):
            vector.wait_ge(scr_sem, 1)
            for i in range(NDVE):
                vector.tensor_copy(sb_scr[0:8, 64 + i:65 + i], sb_scr[0:8, 0:1])
            # one [128, 4*cap] eviction per quad per u-chunk
            for q in range(2):
                for b in range(2):
                    vector.wait_ge(mm_sem, 8 * q + 4 * b + 4)
                    vector.tensor_copy(
                        sb_out[b][:, 4 * q * cap:(4 * q + 4) * cap],
                        ps[q][b][:, 0:4 * cap],
                    ).then_inc(cp_sem, 1)

    return nc


def _route(content_idx: np.ndarray, x: np.ndarray, cap: int):
    """Sort samples by expert; compute per-core padded packed-x shards.

    Returns xt_all in the device DMA layout [NCORES, 128, 2, EPC*cap]
    (partition p = d % 128, K-chunk k = d // 128), fp16.
    """
    idx = content_idx.reshape(-1).astype(np.int64)
    order = np.argsort(idx, kind="stable")
    e_sorted = idx[order]
    counts = np.bincount(idx, minlength=C)
    while counts.max() > cap:
        cap *= 2
    start = np.zeros(C, dtype=np.int64)
    start[1:] = np.cumsum(counts)[:-1]
    slot = np.arange(B) - start[e_sorted]
    core = e_sorted // EPC
    jl = e_sorted % EPC
    # xt columns are in local-expert order (matmul j reads block j); the
    # transposed device out shares the same column index
    xcol = jl * cap + slot

    xt_all = np.zeros((NCORES, 128, 2, EPC * cap), dtype=np.float16)
    # sample vector (256,) -> [k, p] -> transpose to [p, k]
    xs = x[order].astype(np.float16).reshape(B, 2, 128).transpose(0, 2, 1)
    xt_all[core, :, :, xcol] = xs
    return cap, order, core, xcol, xt_all


def _unshard(outs: np.ndarray, order, core, col, cap: int) -> np.ndarray:
    """Scatter per-core transposed device output back to sample order.

    outs: [NCORES, 2, 128, EPC*cap] fp16 -> out [B, U] f32 with
    u = b*128 + p.
    """
    out_full = np.empty((B, U), dtype=np.float32)
    out_full[order] = (
        outs[core, :, :, col].reshape(B, U).astype(np.float32)
    )
    return out_full


def _make_in_maps(xt_all: np.ndarray, kernel_w: np.ndarray):
    # [C, D, U] -> [NC, EPC, 2, 128, U] -> [NC, 128, (e k u)] — per-
    # partition linear, so any contiguous expert range is one DMA slice
    w = np.ascontiguousarray(
        kernel_w.astype(np.float16)
        .reshape(NCORES, EPC, 2, 128, U)
        .transpose(0, 3, 1, 2, 4)
        .reshape(NCORES, 128, EPC * 2 * U)
    )
    xt = xt_all.reshape(NCORES, 128, -1)
    return [{"xt": xt[c], "w": w[c]} for c in range(NCORES)]


def kernel(content_idx: np.ndarray, x: np.ndarray, kernel: np.ndarray) -> np.ndarray:
    from concourse.bass_utils import run_bass_kernel_spmd

    cap, order, core, col, xt_all = _route(content_idx, x, CAP)
    if cap > CAP:
        # Pathologically skewed routing (an expert holds >CAP samples) can't
        # use the static packed program. Unreachable for the fixed-seed
        # problem data; fall back to a host computation to stay correct.
        idx = content_idx.reshape(-1).astype(np.int64)
        return np.einsum("bd,bdu->bu", x.astype(np.float32),
                         kernel.astype(np.float32)[idx]).astype(np.float32)

    key = (cap, 1)
    if key not in _prog_cache:
        _prog_cache[key] = _build_program(cap, 1)
    nc = _prog_cache[key]

    in_maps = _make_in_maps(xt_all, kernel)
    res = run_bass_kernel_spmd(nc, in_maps, list(range(NCORES)))
    outs = np.stack([res.results[c]["out"] for c in range(NCORES)])
    return _unshard(outs, order, core, col, cap)
